# revision 16
# baseline (speedup 1.0000x reference)
"""AvgPool2d-as-Toeplitz kernel for Trainium2 (8 NeuronCores, SPMD).

Reference computes out = (enc_x * mask) @ W.T where W is the dense
Toeplitz matrix of conv2d with kernel ones(C,C,KH,KW)/(KH*KW) over the
flattened zero-padded input (C=16, KH=KW=2, stride 2, pad 1, H=W=32),
and mask zeroes the 1-pixel padding ring of each 34x34 channel image.

Structure exploited:
  W[(co,oi,oj), (ci,i,j)] = 0.25  iff  i in {2oi, 2oi+1} and j in {2oj, 2oj+1}
— independent of co, summed over every ci. Hence with x viewed as
[B, C, 34, 34] and the mask ring folded in structurally (pooling windows
simply never read the masked border rows/columns):

  out[b, co, oi, oj] = 0.25 * sum_ci sum_window x[b, ci, i, j]
       over i in {2oi, 2oi+1} ∩ [1,32],  j in {2oj, 2oj+1} ∩ [1,32]

i.e. one channel-summed 2x2/stride-2 pooled [17,17] map per batch,
replicated across the 16 output channels. ~2.4 MB of input instead of
the 342 MB dense weight + 2.4 MB mask.

Per-core plan (4 batches per core, batch-parallel across 8 cores):
  partitions = (b, ci) = 64, free = flattened 34x34 channel image.
  0. GPSIMD builds E[(b,ci), (b2,co)] = 0.25*(b==b2) with 5 memsets
     (runs under the input DMA).
  1. x-shard [64, 1156] DMA'd in two halves (image rows 0-16 / 17-33)
     on the two HWDGE rings (sync + scalar engines).
  2. Column-pair stage -> a[p, i, oj] (i: 34, oj: 17), DVE:
       a[:, :, 1:16]   = x[:, :, 2:32:2] + x[:, :, 3:33:2]   (two halves)
       a[:, :, {0,16}] = x[:, :, {1,32}]     (masked border cols, 1 copy)
  3. Row-pair stage -> a2[p, oi, oj], DVE:
       a2[:, 1:16, :]   = a[:, 2:32:2, :] + a[:, 3:33:2, :]
       a2[:, {0,16}, :] = a[:, {1,32}, :]    (masked border rows, 1 copy)
  4. PE: single matmul  psum[(b,co), (oi,oj)] = E.T @ a2
     performing the ci-sum, the 0.25 scale and the broadcast over co.
  5. DVE copy PSUM -> SBUF, single DMA to out[4, 4624].
"""

import sys

import numpy as np

if "/opt/trn_rl_repo" not in sys.path:
    sys.path.insert(0, "/opt/trn_rl_repo")

B, C = 32, 16
HP = WP = 34
OH = OW = 17
IMG = HP * WP             # 1156
IN_DIM = C * IMG          # 18496
OUT_DIM = C * OH * OW     # 4624
N_CORES = 8
B_SH = B // N_CORES       # 4 batches per core
P = B_SH * C              # 64 partitions in use
ROWS0 = 17                # rows in first DMA half

_PROGRAM = None


def _build_program():
    import concourse.bacc as bacc
    import concourse.mybir as mybir
    from concourse.tile import TileContext

    f32 = mybir.dt.float32
    nc = bacc.Bacc()

    x = nc.declare_dram_parameter("x", [B_SH, IN_DIM], f32, isOutput=False)
    out = nc.declare_dram_parameter("out", [B_SH, OUT_DIM], f32, isOutput=True)
    xv = x[:, :].rearrange("b (c f) -> (b c) f", c=C)   # [64, 1156]

    with TileContext(nc) as tc:
        with (
            tc.tile_pool(name="sb", bufs=1) as pool,
            tc.tile_pool(name="ps", bufs=1, space="PSUM") as psum_pool,
        ):
            # E built on-device while the DMA is in flight:
            # E[p, (qb,qc)] = 0.25 iff p//16 == qb, i.e. 0 <= p - 16*qb <= 15
            et = pool.tile([P, P], f32)
            nc.gpsimd.memset(et[:], 0.25)
            e3 = et[:].rearrange("p (qb qc) -> p qb qc", qb=B_SH)
            nc.gpsimd.affine_select(
                e3, e3, [[-C, B_SH], [0, C]], mybir.AluOpType.is_ge, 0.0,
                base=0, channel_multiplier=1,
            )
            nc.gpsimd.affine_select(
                e3, e3, [[C, B_SH], [0, C]], mybir.AluOpType.is_ge, 0.0,
                base=C - 1, channel_multiplier=-1,
            )

            xt = pool.tile([P, IMG], f32)
            FH = ROWS0 * WP  # 578
            nc.scalar.dma_start(xt[:, 0:FH], xv[:, 0:FH])
            nc.sync.dma_start(xt[:, FH:IMG], xv[:, FH:IMG])

            x3 = xt[:].rearrange("p (i j) -> p i j", i=HP)

            # column-pair stage: a[p, i, oj]
            at = pool.tile([P, HP * OW], f32)
            a3 = at[:].rearrange("p (i oj) -> p i oj", i=HP)
            nc.vector.tensor_tensor(
                a3[:, 0:ROWS0, 1:16],
                x3[:, 0:ROWS0, 2:32:2], x3[:, 0:ROWS0, 3:33:2],
                mybir.AluOpType.add,
            )
            nc.vector.tensor_tensor(
                a3[:, ROWS0:HP, 1:16],
                x3[:, ROWS0:HP, 2:32:2], x3[:, ROWS0:HP, 3:33:2],
                mybir.AluOpType.add,
            )
            # border cols 0 and 16 <- x cols 1 and 32, one strided copy
            nc.vector.tensor_copy(a3[:, :, 0:17:16], x3[:, :, 1:33:31])

            # row-pair stage: a2[p, oi, oj]
            a2t = pool.tile([P, OH * OW], f32)
            a23 = a2t[:].rearrange("p (oi oj) -> p oi oj", oi=OH)
            nc.vector.tensor_tensor(
                a23[:, 1:16, :], a3[:, 2:32:2, :], a3[:, 3:33:2, :],
                mybir.AluOpType.add,
            )
            # border rows 0 and 16 <- a rows 1 and 32, one strided copy
            nc.vector.tensor_copy(a23[:, 0:17:16, :], a3[:, 1:33:31, :])

            # ci-sum + 0.25 + co-broadcast via PE
            pt = psum_pool.tile([P, OH * OW], f32)
            nc.tensor.matmul(pt[:], et[:], a2t[:], start=True, stop=True)

            # split PSUM->SBUF copy + store across both HWDGE rings so the
            # second copy overlaps the first DMA's issue latency
            ov = out[:, :].rearrange("b (co s) -> (b co) s", co=C)
            ot = pool.tile([P, OH * OW], f32)
            H1 = 145
            nc.vector.tensor_copy(ot[:, 0:H1], pt[:, 0:H1])
            nc.sync.dma_start(ov[:, 0:H1], ot[:, 0:H1])
            nc.vector.tensor_copy(ot[:, H1:], pt[:, H1:])
            nc.scalar.dma_start(ov[:, H1:], ot[:, H1:])
    nc.compile()
    return nc


def _get_program():
    global _PROGRAM
    if _PROGRAM is None:
        _PROGRAM = _build_program()
    return _PROGRAM


def _run(enc_x: np.ndarray, mask: np.ndarray = None, **spmd_kwargs):
    from concourse.bass_utils import run_bass_kernel_spmd

    nc = _get_program()
    in_maps = []
    for i in range(N_CORES):
        sl = slice(i * B_SH, (i + 1) * B_SH)
        in_maps.append({"x": np.ascontiguousarray(enc_x[sl], dtype=np.float32)})
    res = run_bass_kernel_spmd(nc, in_maps, list(range(N_CORES)), **spmd_kwargs)
    out = np.concatenate([res.results[i]["out"] for i in range(N_CORES)], axis=0)
    return out, res


def kernel(enc_x, weight=None, mask=None, **_unused):
    enc_x = np.asarray(enc_x, dtype=np.float32)
    assert enc_x.shape == (B, IN_DIM), enc_x.shape
    out, _ = _run(enc_x)
    return out


# revision 17
# speedup vs baseline: 1.0103x; 1.0103x over previous
"""AvgPool2d-as-Toeplitz kernel for Trainium2 (8 NeuronCores, SPMD).

Reference computes out = (enc_x * mask) @ W.T where W is the dense
Toeplitz matrix of conv2d with kernel ones(C,C,KH,KW)/(KH*KW) over the
flattened zero-padded input (C=16, KH=KW=2, stride 2, pad 1, H=W=32),
and mask zeroes the 1-pixel padding ring of each 34x34 channel image.

Structure exploited:
  W[(co,oi,oj), (ci,i,j)] = 0.25  iff  i in {2oi, 2oi+1} and j in {2oj, 2oj+1}
— independent of co, summed over every ci. Hence with x viewed as
[B, C, 34, 34] and the mask ring folded in structurally (pooling windows
simply never read the masked border rows/columns):

  out[b, co, oi, oj] = 0.25 * sum_ci sum_window x[b, ci, i, j]
       over i in {2oi, 2oi+1} ∩ [1,32],  j in {2oj, 2oj+1} ∩ [1,32]

i.e. one channel-summed 2x2/stride-2 pooled [17,17] map per batch,
replicated across the 16 output channels. ~2.4 MB of input instead of
the 342 MB dense weight + 2.4 MB mask.

Per-core plan (4 batches per core, batch-parallel across 8 cores),
raw bacc with manual semaphores (no Tile pool/tail barriers):
  partitions = (b, ci) = 64, free = flattened 34x34 channel image.
  GPS : E[(b,ci),(b2,co)] = 0.25*(b==b2) via memset + 2 affine_selects
        (runs under the input DMA).
  ACT : DMA image rows 0-16   (HWDGE ring qActDynamicHW)
  SP  : DMA image rows 17-33  (HWDGE ring qSyncDynamicHW)
  DVE : column-pair adds (two row-halves), border-column copy,
        row-pair add, border-row copy
  PE  : psum[(b,co),(oi,oj)] = E.T @ a2   (ci-sum + 0.25 + co-broadcast)
  DVE : copy PSUM -> SBUF
  ACT : DMA out [4, 4624], wait for completion
"""

import sys

import numpy as np

if "/opt/trn_rl_repo" not in sys.path:
    sys.path.insert(0, "/opt/trn_rl_repo")

B, C = 32, 16
HP = WP = 34
OH = OW = 17
IMG = HP * WP             # 1156
IN_DIM = C * IMG          # 18496
OUT_DIM = C * OH * OW     # 4624
N_CORES = 8
B_SH = B // N_CORES       # 4 batches per core
P = B_SH * C              # 64 partitions in use
ROWS0 = 17                # rows in first DMA half
FH = ROWS0 * WP           # 578

_PROGRAM = None


def _build_program():
    import concourse.bacc as bacc
    import concourse.mybir as mybir
    from concourse.tile import TileContext  # noqa: F401  (env parity)

    f32 = mybir.dt.float32
    add = mybir.AluOpType.add
    nc = bacc.Bacc()

    x = nc.declare_dram_parameter("x", [B_SH, IN_DIM], f32, isOutput=False)
    out = nc.declare_dram_parameter("out", [B_SH, OUT_DIM], f32, isOutput=True)
    xv = x[:, :].rearrange("b (c f) -> (b c) f", c=C)   # [64, 1156]
    ov = out[:, :].rearrange("b (co s) -> (b co) s", co=C)

    with (
        nc.sbuf_tensor([P, IMG], f32) as xt,
        nc.sbuf_tensor([P, P], f32) as et,
        nc.sbuf_tensor([P, HP * OW], f32) as at,
        nc.sbuf_tensor([P, OH * OW], f32) as a2t,
        nc.sbuf_tensor([P, OH * OW], f32) as ot,
        nc.psum_tensor([P, OH * OW], f32) as pt,
        nc.semaphore("s_dma0") as s_dma0,
        nc.semaphore("s_dma1") as s_dma1,
        nc.semaphore("s_gps") as s_gps,
        nc.semaphore("s_dve") as s_dve,
        nc.semaphore("s_pe") as s_pe,
        nc.semaphore("s_out") as s_out,
        nc.Block() as block,
    ):
        x3 = xt[:].rearrange("p (i j) -> p i j", i=HP)
        a3 = at[:].rearrange("p (i oj) -> p i oj", i=HP)
        a23 = a2t[:].rearrange("p (oi oj) -> p oi oj", oi=OH)
        e3 = et[:].rearrange("p (qb qc) -> p qb qc", qb=B_SH)

        @block.scalar
        def _(scalar):
            # rows 0-16 on the ACT HWDGE ring
            scalar.dma_start(xt[:, 0:FH], xv[:, 0:FH]).then_inc(s_dma0, 16)
            # out DMA after the PSUM->SBUF copy
            scalar.wait_ge(s_dve, 6)
            scalar.dma_start(ov[:], ot[:]).then_inc(s_out, 16)
            scalar.wait_ge(s_out, 16)

        @block.sync
        def _(sync):
            # rows 17-33 on the SP HWDGE ring
            sync.dma_start(xt[:, FH:IMG], xv[:, FH:IMG]).then_inc(s_dma1, 16)

        @block.gpsimd
        def _(gpsimd):
            # E[p,(qb,qc)] = 0.25 iff 0 <= p - 16*qb <= 15
            gpsimd.memset(et[:], 0.25)
            nc.gpsimd.affine_select(
                e3, e3, [[-C, B_SH], [0, C]], mybir.AluOpType.is_ge, 0.0,
                base=0, channel_multiplier=1,
            )
            nc.gpsimd.affine_select(
                e3, e3, [[C, B_SH], [0, C]], mybir.AluOpType.is_ge, 0.0,
                base=C - 1, channel_multiplier=-1,
            ).then_inc(s_gps, 1)

        @block.vector
        def _(vector):
            vector.wait_ge(s_dma0, 16)
            nc.vector.tensor_tensor(
                a3[:, 0:ROWS0, 1:16],
                x3[:, 0:ROWS0, 2:32:2], x3[:, 0:ROWS0, 3:33:2], add,
            ).then_inc(s_dve, 1)
            vector.wait_ge(s_dma1, 16)
            nc.vector.tensor_tensor(
                a3[:, ROWS0:HP, 1:16],
                x3[:, ROWS0:HP, 2:32:2], x3[:, ROWS0:HP, 3:33:2], add,
            ).then_inc(s_dve, 1)
            nc.vector.tensor_copy(
                a3[:, :, 0:17:16], x3[:, :, 1:33:31]
            ).then_inc(s_dve, 1)
            nc.vector.tensor_tensor(
                a23[:, 1:16, :], a3[:, 2:32:2, :], a3[:, 3:33:2, :], add,
            ).then_inc(s_dve, 1)
            nc.vector.tensor_copy(
                a23[:, 0:17:16, :], a3[:, 1:33:31, :]
            ).then_inc(s_dve, 1)
            vector.wait_ge(s_pe, 1)
            nc.vector.tensor_copy(ot[:], pt[:]).then_inc(s_dve, 1)

        @block.tensor
        def _(tensor):
            tensor.wait_ge(s_dve, 5)
            tensor.wait_ge(s_gps, 1)
            nc.tensor.matmul(
                pt[:], et[:], a2t[:], start=True, stop=True
            ).then_inc(s_pe, 1)

    nc.compile()
    return nc


def _get_program():
    global _PROGRAM
    if _PROGRAM is None:
        _PROGRAM = _build_program()
    return _PROGRAM


def _run(enc_x: np.ndarray, mask: np.ndarray = None, **spmd_kwargs):
    from concourse.bass_utils import run_bass_kernel_spmd

    nc = _get_program()
    in_maps = []
    for i in range(N_CORES):
        sl = slice(i * B_SH, (i + 1) * B_SH)
        in_maps.append({"x": np.ascontiguousarray(enc_x[sl], dtype=np.float32)})
    res = run_bass_kernel_spmd(nc, in_maps, list(range(N_CORES)), **spmd_kwargs)
    out = np.concatenate([res.results[i]["out"] for i in range(N_CORES)], axis=0)
    return out, res


def kernel(enc_x, weight=None, mask=None, **_unused):
    enc_x = np.asarray(enc_x, dtype=np.float32)
    assert enc_x.shape == (B, IN_DIM), enc_x.shape
    out, _ = _run(enc_x)
    return out
